# revision 1
# baseline (speedup 1.0000x reference)
"""YOLO-head decode (nms_detection) Bass kernel for 8 trn2 NeuronCores.

Reference computation per pyramid level p [S, S, 3, 85]:
  conf = p[...,0]
  x = (sigmoid(p[...,1]) + i) / S        (i = index along FIRST spatial axis)
  y = (sigmoid(p[...,2]) + j) / S
  w = exp(p[...,3]) * anchor_w           (anchor = pre_scale[dect]/416)
  h = exp(p[...,4]) * anchor_h
  lix = argmax(p[...,5:85])  (first-max tie-break)
  row = [x,y,w,h,lix,conf] * (conf > 0.5)
Output = concat over levels of rows, [681408, 6].

Sharding: each level split along its leading spatial axis into 8 row-shards
(104->13, 208->26, 416->52 rows per core). Decode is elementwise per cell, so
cores are fully independent; host concatenates the per-core outputs.

On-chip layout: cells on partitions (k=39 consecutive cells per partition per
tile). Per-cell constants (i/S, j/S, anchor_w, anchor_h) are precomputed on
host into a [N,4] aux tensor. argmax is computed exactly in fp32:
  max-tree (scalar_tensor_tensor max) -> is_equal vs broadcast max
  -> * (79 - class) iota -> max-tree -> lix = 79 - result
which reproduces jnp.argmax's first-index tie-break exactly.
"""

import os
import sys
from contextlib import ExitStack

import numpy as np

for _p in ("/root/.axon_site/_ro/trn_rl_repo", "/opt/trn_rl_repo"):
    if os.path.isdir(_p) and _p not in sys.path:
        sys.path.append(_p)

import concourse.bacc as bacc
import concourse.bass as bass
import concourse.tile as tile
import concourse.mybir as mybir
from concourse.bass_utils import run_bass_kernel_spmd

F32 = mybir.dt.float32
Alu = mybir.AluOpType
Act = mybir.ActivationFunctionType

N_CORES = 8
K = 39  # cells per partition per tile

# (name, S, rows_per_core, dect_size, partitions, tiles_per_core)
LEVELS = [
    ("small", 104, 13, 3, 104, 1),
    ("middle", 208, 26, 4, 104, 4),
    ("large", 416, 52, 5, 128, 13),
]

LAST_EXEC_NS = None
LAST_RESULTS = None

_prog_cache = {}


def _stt(eng, out, in0, in1, op1, scalar=1.0, op0=Alu.mult):
    """out = (in0 op0 scalar) op1 in1"""
    eng.scalar_tensor_tensor(out, in0, scalar, in1, op0, op1)


def _tree_max(eng, v):
    """In-place max-reduce of [P, K, 80] view down to v[:, :, 0:1]."""
    _stt(eng, v[:, :, 0:40], v[:, :, 0:40], v[:, :, 40:80], Alu.max)
    for w in (20, 10, 5):
        _stt(eng, v[:, :, 0:w], v[:, :, 0:w], v[:, :, w : 2 * w], Alu.max)
    # 5 left: (0,1)x(2,3) -> (0,1); 0x1 -> 0; 0x4 -> 0
    _stt(eng, v[:, :, 0:2], v[:, :, 0:2], v[:, :, 2:4], Alu.max)
    _stt(eng, v[:, :, 0:1], v[:, :, 0:1], v[:, :, 1:2], Alu.max)
    _stt(eng, v[:, :, 0:1], v[:, :, 0:1], v[:, :, 4:5], Alu.max)


def _build_program():
    nc = bacc.Bacc(trn_type="TRN2")
    xins, auxs, outs = {}, {}, {}
    for nm, S, rows, dect, P, T in LEVELS:
        N = rows * S * 3
        xins[nm] = nc.dram_tensor(f"x_{nm}", [N, 85], F32, kind="ExternalInput")
        auxs[nm] = nc.dram_tensor(f"a_{nm}", [N, 4], F32, kind="ExternalInput")
        outs[nm] = nc.dram_tensor(f"o_{nm}", [N, 6], F32, kind="ExternalOutput")
    iot = nc.dram_tensor("iota", [128, 80], F32, kind="ExternalInput")

    with tile.TileContext(nc) as tc, ExitStack() as ctx:
        const = ctx.enter_context(tc.tile_pool(name="const", bufs=1))
        pin_p = ctx.enter_context(tc.tile_pool(name="pin", bufs=3))
        aux_p = ctx.enter_context(tc.tile_pool(name="auxp", bufs=3))
        scr_p = ctx.enter_context(tc.tile_pool(name="scr", bufs=2))
        eq_p = ctx.enter_context(tc.tile_pool(name="eqp", bufs=2))
        sml_p = ctx.enter_context(tc.tile_pool(name="sml", bufs=3))
        out_p = ctx.enter_context(tc.tile_pool(name="outp", bufs=3))

        it = const.tile([128, 80], F32)
        nc.sync.dma_start(it[:], iot[:])

        for nm, S, rows, dect, P, T in LEVELS:
            inv = float(np.float32(1.0 / S))
            xv = xins[nm][:].rearrange("(t p k) c -> t p (k c)", p=P, k=K)
            av = auxs[nm][:].rearrange("(t p k) c -> t p (k c)", p=P, k=K)
            ov = outs[nm][:].rearrange("(t p k) c -> t p (k c)", p=P, k=K)
            iota_b = (
                it[0:P, :].rearrange("p (o c) -> p o c", o=1).broadcast_to([P, K, 80])
            )
            for t in range(T):
                pin = pin_p.tile([P, K * 85], F32, tag="pin")
                nc.sync.dma_start(pin[:], xv[t])
                axt = aux_p.tile([P, K * 4], F32, tag="aux")
                nc.sync.dma_start(axt[:], av[t])
                pv = pin[:].rearrange("p (k c) -> p k c", c=85)
                axv = axt[:].rearrange("p (k c) -> p k c", c=4)

                ot = out_p.tile([P, K * 6], F32, tag="out")
                ovv = ot[:].rearrange("p (k c) -> p k c", c=6)

                # ACT engine: transcendentals + conf copy
                sg = sml_p.tile([P, K * 2], F32, tag="sig")
                sgv = sg[:].rearrange("p (k c) -> p k c", c=2)
                nc.scalar.activation(sgv, pv[:, :, 1:3], Act.Sigmoid)
                ext = sml_p.tile([P, K * 2], F32, tag="exp")
                exv = ext[:].rearrange("p (k c) -> p k c", c=2)
                nc.scalar.activation(exv, pv[:, :, 3:5], Act.Exp)
                nc.scalar.copy(ovv[:, :, 5:6], pv[:, :, 0:1])

                # class max (exact fp32 tree)
                t40 = scr_p.tile([P, K * 40], F32, tag="t40")
                tv = t40[:].rearrange("p (k c) -> p k c", c=40)
                _stt(nc.vector, tv[:, :, 0:40], pv[:, :, 5:45], pv[:, :, 45:85], Alu.max)
                for w in (20, 10, 5):
                    _stt(nc.vector, tv[:, :, 0:w], tv[:, :, 0:w], tv[:, :, w : 2 * w], Alu.max)
                _stt(nc.vector, tv[:, :, 0:2], tv[:, :, 0:2], tv[:, :, 2:4], Alu.max)
                _stt(nc.vector, tv[:, :, 0:1], tv[:, :, 0:1], tv[:, :, 1:2], Alu.max)
                _stt(nc.vector, tv[:, :, 0:1], tv[:, :, 0:1], tv[:, :, 4:5], Alu.max)
                m_b = tv[:, :, 0:1].broadcast_to([P, K, 80])

                # eq = (V == m); imx = eq * (79 - c); idx = max(imx)
                eq = eq_p.tile([P, K * 80], F32, tag="eq")
                eqv = eq[:].rearrange("p (k c) -> p k c", c=80)
                _stt(nc.vector, eqv, pv[:, :, 5:85], m_b, Alu.is_equal)
                _stt(nc.vector, eqv, eqv, iota_b, Alu.mult)
                _tree_max(nc.vector, eqv)
                # lix = 79 - idx
                nc.vector.tensor_scalar(
                    ovv[:, :, 4:5], eqv[:, :, 0:1], -1.0, 79.0, Alu.mult, Alu.add
                )

                # mask = conf > 0.5
                mk = sml_p.tile([P, K], F32, tag="mk")
                nc.vector.tensor_scalar(mk[:], pv[:, :, 0], 0.5, None, Alu.is_gt)

                # x = sig*inv + gx ; y = sig*inv + gy
                _stt(nc.vector, ovv[:, :, 0:1], sgv[:, :, 0:1], axv[:, :, 0:1], Alu.add, scalar=inv)
                _stt(nc.vector, ovv[:, :, 1:2], sgv[:, :, 1:2], axv[:, :, 1:2], Alu.add, scalar=inv)
                # w = exp*aw ; h = exp*ah
                _stt(nc.vector, ovv[:, :, 2:3], exv[:, :, 0:1], axv[:, :, 2:3], Alu.mult)
                _stt(nc.vector, ovv[:, :, 3:4], exv[:, :, 1:2], axv[:, :, 3:4], Alu.mult)

                # zero failing rows
                mk_b = mk[:].rearrange("p (k o) -> p k o", o=1).broadcast_to([P, K, 6])
                _stt(nc.vector, ovv, ovv, mk_b, Alu.mult)

                nc.sync.dma_start(ov[t], ot[:])
    nc.compile()
    return nc


def _get_program():
    if "nc" not in _prog_cache:
        _prog_cache["nc"] = _build_program()
    return _prog_cache["nc"]


def _make_in_maps(small, middle, large, pre_scale):
    arrs = {"small": small, "middle": middle, "large": large}
    ps = np.asarray(pre_scale, dtype=np.float32)
    iota = np.ascontiguousarray(
        np.broadcast_to(79.0 - np.arange(80, dtype=np.float32), (128, 80))
    )
    in_maps = []
    for c in range(N_CORES):
        m = {"iota": iota}
        for nm, S, rows, dect, P, T in LEVELS:
            N = rows * S * 3
            x = np.asarray(arrs[nm][c * rows : (c + 1) * rows], dtype=np.float32)
            m[f"x_{nm}"] = np.ascontiguousarray(x.reshape(N, 85))
            inv = np.float32(1.0 / S)
            anc = (ps[dect] / np.float32(416.0)).astype(np.float32)  # [3,2]
            a = np.empty((rows, S, 3, 4), np.float32)
            a[..., 0] = (
                np.arange(c * rows, (c + 1) * rows, dtype=np.float32) * inv
            )[:, None, None]
            a[..., 1] = (np.arange(S, dtype=np.float32) * inv)[None, :, None]
            a[..., 2] = anc[:, 0][None, None, :]
            a[..., 3] = anc[:, 1][None, None, :]
            m[f"a_{nm}"] = a.reshape(N, 4)
        in_maps.append(m)
    return in_maps


def kernel(small, middle, large, pre_scale):
    global LAST_EXEC_NS, LAST_RESULTS
    small = np.asarray(small, dtype=np.float32)
    middle = np.asarray(middle, dtype=np.float32)
    large = np.asarray(large, dtype=np.float32)
    in_maps = _make_in_maps(small, middle, large, pre_scale)
    nc = _get_program()
    res = run_bass_kernel_spmd(nc, in_maps, list(range(N_CORES)))
    LAST_EXEC_NS = res.exec_time_ns
    LAST_RESULTS = res
    chunks = []
    for nm, S, rows, dect, P, T in LEVELS:
        for c in range(N_CORES):
            chunks.append(np.asarray(res.results[c][f"o_{nm}"]))
    return np.concatenate(chunks, axis=0)



# revision 9
# speedup vs baseline: 1.6532x; 1.6532x over previous
"""YOLO-head decode (nms_detection) Bass kernel for 8 trn2 NeuronCores.

Reference computation per pyramid level p [S, S, 3, 85]:
  conf = p[...,0]
  x = (sigmoid(p[...,1]) + i) / S        (i = index along FIRST spatial axis)
  y = (sigmoid(p[...,2]) + j) / S
  w = exp(p[...,3]) * anchor_w           (anchor = pre_scale[dect]/416)
  h = exp(p[...,4]) * anchor_h
  lix = argmax(p[...,5:85])  (first-max tie-break)
  row = [x,y,w,h,lix,conf] * (conf > 0.5)
Output = concat over levels of rows, [681408, 6].

Sharding: each level split along its leading spatial axis into 8 row-shards
(104->13, 208->26, 416->52 rows per core). Decode is elementwise per cell, so
cores are fully independent; host concatenates the per-core outputs.

On-chip layout: cells on partitions (K=39 consecutive cells per partition per
tile; a partition's 39 cells never cross an image row since 3*S is always a
multiple of 39, so grid x/y offsets are per-(partition, tile) constants
computed on host into tiny per-level const tables instead of a per-cell aux
tensor).

argmax via value/index packing so one max-reduce yields both:
  ACT:  t   = Identity(v * 2^14 + 1.5*2^23)      -> fp32 magic-round, t = M + r
  Pool: key = (t - M) + ((79-c)/128 - 0.3125)    -> r + (m-40)/128, m = 79-c
  DVE:  kmax = reduce_max(key)  (single InstTensorReduce over the 80 classes)
  ACT:  t2  = Identity(kmax + M)                 -> M + r*
  DVE:  e   = (t2 - M) - kmax = -(m*-40)/128 ;  lix = 128*e + 39 = c*
All steps are exact in fp32 for |v| < 8 (keys are integers r plus exact
multiples of 1/128, |r| <= 2^17), so the only deviation from jnp.argmax is
the 2^-14 quantization of logits (ties broken toward the first index, same
as argmax; measured rel-err contribution ~5e-3, well under the 2e-2 gate).
This replaces the old exact eq/iota/double-max-tree (~320 DVE elems/cell)
with ~93 DVE + 80 ACT + 80 Pool elems/cell, spread across three engines.
"""

import os
import sys
from contextlib import ExitStack

import numpy as np

for _p in ("/root/.axon_site/_ro/trn_rl_repo", "/opt/trn_rl_repo"):
    if os.path.isdir(_p) and _p not in sys.path:
        sys.path.append(_p)

import concourse.bacc as bacc
import concourse.bass as bass
import concourse.tile as tile
import concourse.mybir as mybir
from concourse.bass_utils import run_bass_kernel_spmd

F32 = mybir.dt.float32
Alu = mybir.AluOpType
Act = mybir.ActivationFunctionType
AxX = mybir.AxisListType.X

N_CORES = 8
K = 39  # cells per partition per tile
MAGIC = 12582912.0  # 1.5 * 2^23: float + MAGIC - MAGIC == round-to-nearest-int
QS = 16384.0  # 2^14 logit quantization scale

# (name, S, rows_per_core, dect_size, partitions, tiles_per_core, gy_variants)
LEVELS = [
    ("small", 104, 13, 3, 104, 1, 1),
    ("middle", 208, 26, 4, 104, 4, 2),
    ("large", 416, 52, 5, 128, 13, 1),
]

LAST_EXEC_NS = None
LAST_RESULTS = None

_prog_cache = {}


def _build_program():
    nc = bacc.Bacc(trn_type="TRN2")
    xins, csts, outs = {}, {}, {}
    for nm, S, rows, dect, P, T, n_var in LEVELS:
        N = rows * S * 3
        W = n_var * K + 2 * K + T
        xins[nm] = nc.dram_tensor(f"x_{nm}", [N, 85], F32, kind="ExternalInput")
        csts[nm] = nc.dram_tensor(f"c_{nm}", [128, W], F32, kind="ExternalInput")
        outs[nm] = nc.dram_tensor(f"o_{nm}", [N, 6], F32, kind="ExternalOutput")
    # col 80 holds MAGIC so activation bias can be passed as a [P,1] AP
    iot = nc.dram_tensor("iota", [128, 81], F32, kind="ExternalInput")

    with tile.TileContext(nc) as tc, ExitStack() as ctx:
        const = ctx.enter_context(tc.tile_pool(name="const", bufs=1))
        pin_p = ctx.enter_context(tc.tile_pool(name="pin", bufs=4))
        qt_p = ctx.enter_context(tc.tile_pool(name="qtp", bufs=2))
        km_p = ctx.enter_context(tc.tile_pool(name="kmp", bufs=2))
        sml_p = ctx.enter_context(tc.tile_pool(name="sml", bufs=3))
        out_p = ctx.enter_context(tc.tile_pool(name="outp", bufs=3))

        it = const.tile([128, 81], F32)
        nc.sync.dma_start(it[:], iot[:])
        ctiles = {}
        for nm, S, rows, dect, P, T, n_var in LEVELS:
            W = n_var * K + 2 * K + T
            ct = const.tile([128, W], F32)
            nc.sync.dma_start(ct[:], csts[nm][:])
            ctiles[nm] = ct

        for nm, S, rows, dect, P, T, n_var in LEVELS:
            inv = float(np.float32(1.0 / S))
            xv = xins[nm][:].rearrange("(t p k) c -> t p (k c)", p=P, k=K)
            ov = outs[nm][:].rearrange("(t p k) c -> t p (k c)", p=P, k=K)
            ct = ctiles[nm]
            awh_off = n_var * K
            gx_off = n_var * K + 2 * K
            iota_b = (
                it[0:P, 0:80]
                .rearrange("p (o c) -> p o c", o=1)
                .broadcast_to([P, K, 80])
            )
            magic_ap = it[0:P, 80:81]
            awh_v = ct[0:P, awh_off : awh_off + 2 * K].rearrange(
                "p (k c) -> p k c", c=2
            )
            for t in range(T):
                v = t % n_var
                gy_v = ct[0:P, v * K : (v + 1) * K].rearrange("p (k c) -> p k c", c=1)
                gx_v = (
                    ct[0:P, gx_off + t : gx_off + t + 1]
                    .rearrange("p (o c) -> p o c", o=1)
                    .broadcast_to([P, K, 1])
                )
                pin = pin_p.tile([P, K * 85], F32, tag="pin")
                nc.sync.dma_start(pin[:], xv[t])
                pv = pin[:].rearrange("p (k c) -> p k c", c=85)

                ot = out_p.tile([P, K * 6], F32, tag="out")
                ovv = ot[:].rearrange("p (k c) -> p k c", c=6)

                # ACT engine: transcendentals, conf copy, logit magic-round
                sg = sml_p.tile([P, K * 2], F32, tag="sig")
                sgv = sg[:].rearrange("p (k c) -> p k c", c=2)
                nc.scalar.activation(sgv, pv[:, :, 1:3], Act.Sigmoid)
                ext = sml_p.tile([P, K * 2], F32, tag="exp")
                exv = ext[:].rearrange("p (k c) -> p k c", c=2)
                nc.scalar.activation(exv, pv[:, :, 3:5], Act.Exp)
                nc.scalar.copy(ovv[:, :, 5:6], pv[:, :, 0:1])
                qt = qt_p.tile([P, K * 80], F32, tag="qt")
                qv = qt[:].rearrange("p (k c) -> p k c", c=80)
                nc.scalar.activation(
                    qv, pv[:, :, 5:85], Act.Identity, bias=magic_ap, scale=QS
                )

                # DVE: key = (t - M) + iota'   (in place over qt; Pool rejects
                # TensorScalarPtr at codegen, so this must live on DVE)
                nc.vector.scalar_tensor_tensor(qv, qv, -MAGIC, iota_b, Alu.add, Alu.add)

                # DVE: packed max over the 80 classes in one reduce
                km = km_p.tile([P, K], F32, tag="km")
                nc.vector.tensor_reduce(km[:], qv, axis=AxX, op=Alu.max)

                # ACT: round kmax back to its integer part (t2 = M + r*)
                t2 = sml_p.tile([P, K], F32, tag="t2")
                nc.scalar.activation(t2[:], km[:], Act.Identity, bias=magic_ap)

                # DVE: e = (t2 - M) - kmax = -f* ; lix = 128*e + 39 (exact int)
                e = sml_p.tile([P, K], F32, tag="e")
                nc.vector.scalar_tensor_tensor(
                    e[:], t2[:], -MAGIC, km[:], Alu.add, Alu.subtract
                )
                ev = e[:].rearrange("p (k c) -> p k c", c=1)
                nc.vector.tensor_scalar(
                    ovv[:, :, 4:5], ev, 128.0, 39.0, Alu.mult, Alu.add
                )

                # DVE: x = sig*inv + gx ; y = sig*inv + gy ; (w,h) = exp * (aw,ah)
                nc.vector.scalar_tensor_tensor(
                    ovv[:, :, 0:1], sgv[:, :, 0:1], inv, gx_v, Alu.mult, Alu.add
                )
                nc.vector.scalar_tensor_tensor(
                    ovv[:, :, 1:2], sgv[:, :, 1:2], inv, gy_v, Alu.mult, Alu.add
                )
                nc.vector.scalar_tensor_tensor(
                    ovv[:, :, 2:4], exv[:, :, 0:2], 1.0, awh_v, Alu.mult, Alu.mult
                )

                # mask = conf > 0.5 ; zero failing rows
                mk = sml_p.tile([P, K], F32, tag="mk")
                nc.vector.tensor_scalar(mk[:], pv[:, :, 0], 0.5, None, Alu.is_gt)
                mk_b = mk[:].rearrange("p (k o) -> p k o", o=1).broadcast_to([P, K, 6])
                nc.vector.scalar_tensor_tensor(ovv, ovv, 1.0, mk_b, Alu.mult, Alu.mult)

                nc.sync.dma_start(ov[t], ot[:])
    nc.compile()
    return nc


def _get_program():
    if "nc" not in _prog_cache:
        _prog_cache["nc"] = _build_program()
    return _prog_cache["nc"]


def _make_cst(core, ps, S, rows, dect, P, T, n_var):
    """Per-level const table [128, n_var*K + 2K + T]: gy variants | awh | gx."""
    inv = np.float32(1.0 / S)
    anc = (ps[dect] / np.float32(416.0)).astype(np.float32)  # [3, 2]
    W = n_var * K + 2 * K + T
    a = np.zeros((128, W), np.float32)
    base = core * rows * S * 3
    p_idx = np.arange(P)[:, None]
    k_idx = np.arange(K)[None, :]
    for v in range(n_var):
        g = base + (v * P + p_idx) * K + k_idx
        j = (g % (S * 3)) // 3
        a[:P, v * K : (v + 1) * K] = j.astype(np.float32) * inv
    # anchors repeat with period 3 along k (39 cells = 13 j-positions x 3)
    a[:P, n_var * K : n_var * K + 2 * K : 2] = anc[np.arange(K) % 3, 0][None, :]
    a[:P, n_var * K + 1 : n_var * K + 2 * K : 2] = anc[np.arange(K) % 3, 1][None, :]
    t_idx = np.arange(T)[None, :]
    g0 = base + (t_idx * P + p_idx) * K
    a[:P, n_var * K + 2 * K :] = (g0 // (S * 3)).astype(np.float32) * inv
    return a


def _make_in_maps(small, middle, large, pre_scale):
    arrs = {"small": small, "middle": middle, "large": large}
    ps = np.asarray(pre_scale, dtype=np.float32)
    iota = np.empty((128, 81), np.float32)
    iota[:, 0:80] = (79.0 - np.arange(80)).astype(np.float32) / np.float32(
        128.0
    ) - np.float32(0.3125)
    iota[:, 80] = MAGIC
    in_maps = []
    for c in range(N_CORES):
        m = {"iota": iota}
        for nm, S, rows, dect, P, T, n_var in LEVELS:
            N = rows * S * 3
            x = np.asarray(arrs[nm][c * rows : (c + 1) * rows], dtype=np.float32)
            m[f"x_{nm}"] = np.ascontiguousarray(x.reshape(N, 85))
            m[f"c_{nm}"] = _make_cst(c, ps, S, rows, dect, P, T, n_var)
        in_maps.append(m)
    return in_maps


def kernel(small, middle, large, pre_scale):
    global LAST_EXEC_NS, LAST_RESULTS
    small = np.asarray(small, dtype=np.float32)
    middle = np.asarray(middle, dtype=np.float32)
    large = np.asarray(large, dtype=np.float32)
    in_maps = _make_in_maps(small, middle, large, pre_scale)
    nc = _get_program()
    res = run_bass_kernel_spmd(nc, in_maps, list(range(N_CORES)))
    LAST_EXEC_NS = res.exec_time_ns
    LAST_RESULTS = res
    chunks = []
    for nm, S, rows, dect, P, T, n_var in LEVELS:
        for c in range(N_CORES):
            chunks.append(np.asarray(res.results[c][f"o_{nm}"]))
    return np.concatenate(chunks, axis=0)


# revision 10
# speedup vs baseline: 1.8533x; 1.1210x over previous
"""YOLO-head decode (nms_detection) Bass kernel for 8 trn2 NeuronCores.

Reference computation per pyramid level p [S, S, 3, 85]:
  conf = p[...,0]
  x = (sigmoid(p[...,1]) + i) / S        (i = index along FIRST spatial axis)
  y = (sigmoid(p[...,2]) + j) / S
  w = exp(p[...,3]) * anchor_w           (anchor = pre_scale[dect]/416)
  h = exp(p[...,4]) * anchor_h
  lix = argmax(p[...,5:85])  (first-max tie-break)
  row = [x,y,w,h,lix,conf] * (conf > 0.5)
Output = concat over levels of rows, [681408, 6].

Sharding: each level split along its leading spatial axis into 8 row-shards
(104->13, 208->26, 416->52 rows per core). Decode is elementwise per cell, so
cores are fully independent; host concatenates the per-core outputs.

Layout: cells on partitions, K cells per partition per tile, padded per level
so all 128 partitions are used (small 1x128x32, middle 2x128x64, large
13x128x39; pad cells are zeros -> conf 0 -> masked, host drops them). Grid
x/y offsets and anchors are per-(tile, partition, k) constants computed on
host into small per-level tables (no per-cell aux tensor DMA).

argmax via value/index packing so one max-reduce yields both:
  ACT:  t   = Identity(v * 2^14 + 1.5*2^23)      -> fp32 magic-round, t = M + r
  DVE:  key = (t - M) + ((79-c)/128 - 0.3125)    -> r + (m-40)/128, m = 79-c
  DVE:  kmax = reduce_max(key)  (single InstTensorReduce over the 80 classes)
  ACT:  t2  = Identity(kmax + M)                 -> M + r*
  DVE:  e   = (t2 - M) - kmax = -(m*-40)/128 ;  lix = 128*e + 39 = c*
All steps are exact in fp32 for |v| < 8 (keys are integers r plus exact
multiples of 1/128, |r| <= 2^17), so the only deviation from jnp.argmax is
the 2^-14 quantization of logits (ties broken toward the first index, same
as argmax; measured rel-err contribution ~3.7e-3, well under the 2e-2 gate).

sigmoid is computed as 1/(1+exp(-x)) (ACT Exp with scale=-1 + Identity(+1),
DVE reciprocal_approx_fast) because no ACT table set holds both sigmoid and
exp -- a native Sigmoid would force two 1.3us table reloads per tile.

Outputs are written bf16 (halves output DMA; x/y/w/h/conf tolerate the 2^-9
rounding, lix <= 79 is exact in bf16) and converted to fp32 on host.
"""

import os
import sys
from contextlib import ExitStack

import numpy as np

for _p in ("/root/.axon_site/_ro/trn_rl_repo", "/opt/trn_rl_repo"):
    if os.path.isdir(_p) and _p not in sys.path:
        sys.path.append(_p)

import concourse.bacc as bacc
import concourse.bass as bass
import concourse.tile as tile
import concourse.mybir as mybir
from concourse.bass_utils import run_bass_kernel_spmd

F32 = mybir.dt.float32
BF16 = mybir.dt.bfloat16
Alu = mybir.AluOpType
Act = mybir.ActivationFunctionType
AxX = mybir.AxisListType.X

N_CORES = 8
MAGIC = 12582912.0  # 1.5 * 2^23: float + MAGIC - MAGIC == round-to-nearest-int
QS = 16384.0  # 2^14 logit quantization scale

# (name, S, rows_per_core, dect_size, P, K, T, A)
#   P*K*T >= rows*S*3 (pad), A = number of distinct anchor k-patterns over t
LEVELS = [
    ("small", 104, 13, 3, 128, 32, 1, 1),
    ("middle", 208, 26, 4, 128, 64, 2, 2),
    ("large", 416, 52, 5, 128, 39, 13, 1),
]

LAST_EXEC_NS = None
LAST_RESULTS = None

_prog_cache = {}


def _build_program():
    nc = bacc.Bacc(trn_type="TRN2")
    xins, csts, outs = {}, {}, {}
    for nm, S, rows, dect, P, K, T, A in LEVELS:
        Ncap = P * K * T
        W = (T + A) * 2 * K
        xins[nm] = nc.dram_tensor(f"x_{nm}", [Ncap, 85], F32, kind="ExternalInput")
        csts[nm] = nc.dram_tensor(f"c_{nm}", [128, W], F32, kind="ExternalInput")
        outs[nm] = nc.dram_tensor(f"o_{nm}", [Ncap, 6], BF16, kind="ExternalOutput")
    # cols 0:80 = (79-c)/128 - 0.3125 ; col 80 = MAGIC (activation bias AP)
    iot = nc.dram_tensor("iota", [128, 81], F32, kind="ExternalInput")

    with tile.TileContext(nc) as tc, ExitStack() as ctx:
        const = ctx.enter_context(tc.tile_pool(name="const", bufs=1))
        pin_p = ctx.enter_context(tc.tile_pool(name="pin", bufs=4))
        qt_p = ctx.enter_context(tc.tile_pool(name="qtp", bufs=2))
        km_p = ctx.enter_context(tc.tile_pool(name="kmp", bufs=2))
        sml_p = ctx.enter_context(tc.tile_pool(name="sml", bufs=3))
        out_p = ctx.enter_context(tc.tile_pool(name="outp", bufs=3))

        it = const.tile([128, 81], F32)
        nc.sync.dma_start(it[:], iot[:])
        ctiles = {}
        for nm, S, rows, dect, P, K, T, A in LEVELS:
            ct = const.tile([128, (T + A) * 2 * K], F32)
            nc.sync.dma_start(ct[:], csts[nm][:])
            ctiles[nm] = ct

        magic_ap = it[0:128, 80:81]
        for nm, S, rows, dect, P, K, T, A in LEVELS:
            inv = float(np.float32(1.0 / S))
            xv = xins[nm][:].rearrange("(t p k) c -> t p (k c)", p=P, k=K)
            ov = outs[nm][:].rearrange("(t p k) c -> t p (k c)", p=P, k=K)
            ct = ctiles[nm]
            iota_b = (
                it[0:P, 0:80]
                .rearrange("p (o c) -> p o c", o=1)
                .broadcast_to([P, K, 80])
            )
            for t in range(T):
                gxy_v = ct[0:P, t * 2 * K : (t + 1) * 2 * K].rearrange(
                    "p (k c) -> p k c", c=2
                )
                ao = (T + (t % A)) * 2 * K
                awh_v = ct[0:P, ao : ao + 2 * K].rearrange("p (k c) -> p k c", c=2)

                pin = pin_p.tile([P, K * 85], F32, tag="pin")
                nc.sync.dma_start(pin[:], xv[t])
                pv = pin[:].rearrange("p (k c) -> p k c", c=85)

                ot = out_p.tile([P, K * 6], BF16, tag="out")
                ovv = ot[:].rearrange("p (k c) -> p k c", c=6)

                # ACT: exp(-txy) for sigmoid, exp(twh), conf copy, magic-round
                es = sml_p.tile([P, K * 2], F32, tag="es")
                esv = es[:].rearrange("p (k c) -> p k c", c=2)
                nc.scalar.activation(esv, pv[:, :, 1:3], Act.Exp, scale=-1.0)
                sp = sml_p.tile([P, K * 2], F32, tag="sp")
                spv = sp[:].rearrange("p (k c) -> p k c", c=2)
                nc.scalar.activation(spv, esv, Act.Identity, bias=1.0)
                ext = sml_p.tile([P, K * 2], F32, tag="exp")
                exv = ext[:].rearrange("p (k c) -> p k c", c=2)
                nc.scalar.activation(exv, pv[:, :, 3:5], Act.Exp)
                nc.scalar.copy(ovv[:, :, 5:6], pv[:, :, 0:1])
                qt = qt_p.tile([P, K * 80], F32, tag="qt")
                qv = qt[:].rearrange("p (k c) -> p k c", c=80)
                nc.scalar.activation(
                    qv, pv[:, :, 5:85], Act.Identity, bias=magic_ap, scale=QS
                )

                # DVE: sigmoid = 1/(1+exp(-x))
                sg = sml_p.tile([P, K * 2], F32, tag="sg")
                nc.vector.reciprocal_approx_fast(sg[:], sp[:])
                sgv = sg[:].rearrange("p (k c) -> p k c", c=2)

                # DVE: key = (t - M) + iota'   (in place over qt)
                nc.vector.scalar_tensor_tensor(qv, qv, -MAGIC, iota_b, Alu.add, Alu.add)

                # DVE: packed max over the 80 classes in one reduce
                km = km_p.tile([P, K], F32, tag="km")
                nc.vector.tensor_reduce(km[:], qv, axis=AxX, op=Alu.max)

                # ACT: round kmax back to its integer part (t2 = M + r*)
                t2 = sml_p.tile([P, K], F32, tag="t2")
                nc.scalar.activation(t2[:], km[:], Act.Identity, bias=magic_ap)

                # DVE: e = (t2 - M) - kmax = -f* ; lix = 128*e + 39 (exact int)
                e = sml_p.tile([P, K], F32, tag="e")
                nc.vector.scalar_tensor_tensor(
                    e[:], t2[:], -MAGIC, km[:], Alu.add, Alu.subtract
                )
                ev = e[:].rearrange("p (k c) -> p k c", c=1)
                nc.vector.tensor_scalar(
                    ovv[:, :, 4:5], ev, 128.0, 39.0, Alu.mult, Alu.add
                )

                # DVE: (x,y) = sig*inv + (gx,gy) ; (w,h) = exp * (aw,ah)
                nc.vector.scalar_tensor_tensor(
                    ovv[:, :, 0:2], sgv, inv, gxy_v, Alu.mult, Alu.add
                )
                nc.vector.scalar_tensor_tensor(
                    ovv[:, :, 2:4], exv, 1.0, awh_v, Alu.mult, Alu.mult
                )

                # mask = conf > 0.5 ; zero failing rows
                mk = sml_p.tile([P, K], BF16, tag="mk")
                nc.vector.tensor_scalar(mk[:], pv[:, :, 0], 0.5, None, Alu.is_gt)
                mk_b = mk[:].rearrange("p (k o) -> p k o", o=1).broadcast_to([P, K, 6])
                nc.vector.scalar_tensor_tensor(ovv, ovv, 1.0, mk_b, Alu.mult, Alu.mult)

                nc.sync.dma_start(ov[t], ot[:])
    nc.compile()
    return nc


def _get_program():
    if "nc" not in _prog_cache:
        _prog_cache["nc"] = _build_program()
    return _prog_cache["nc"]


def _make_cst(core, ps, S, rows, dect, P, K, T, A):
    """Per-level const table [128, (T+A)*2K]: per-tile (gx,gy) | awh variants."""
    inv = np.float32(1.0 / S)
    anc = (ps[dect] / np.float32(416.0)).astype(np.float32)  # [3, 2]
    cells = rows * S * 3
    Ncap = P * K * T
    base = core * cells
    g = base + np.minimum(np.arange(Ncap), cells - 1)
    arr = g.reshape(T, P, K)
    i = arr // (S * 3)
    j = (arr % (S * 3)) // 3
    aa = arr % 3
    cst = np.zeros((128, (T + A) * 2 * K), np.float32)
    blk = np.empty((P, K, 2), np.float32)
    for t in range(T):
        blk[..., 0] = i[t].astype(np.float32) * inv
        blk[..., 1] = j[t].astype(np.float32) * inv
        cst[:P, t * 2 * K : (t + 1) * 2 * K] = blk.reshape(P, 2 * K)
    for va in range(A):
        blk[..., 0] = anc[aa[va], 0]
        blk[..., 1] = anc[aa[va], 1]
        cst[:P, (T + va) * 2 * K : (T + va + 1) * 2 * K] = blk.reshape(P, 2 * K)
    return cst


def _make_in_maps(small, middle, large, pre_scale):
    arrs = {"small": small, "middle": middle, "large": large}
    ps = np.asarray(pre_scale, dtype=np.float32)
    iota = np.empty((128, 81), np.float32)
    iota[:, 0:80] = (79.0 - np.arange(80)).astype(np.float32) / np.float32(
        128.0
    ) - np.float32(0.3125)
    iota[:, 80] = MAGIC
    in_maps = []
    for c in range(N_CORES):
        m = {"iota": iota}
        for nm, S, rows, dect, P, K, T, A in LEVELS:
            cells = rows * S * 3
            Ncap = P * K * T
            x = np.asarray(arrs[nm][c * rows : (c + 1) * rows], dtype=np.float32)
            xp = np.zeros((Ncap, 85), np.float32)
            xp[:cells] = x.reshape(cells, 85)
            m[f"x_{nm}"] = xp
            m[f"c_{nm}"] = _make_cst(c, ps, S, rows, dect, P, K, T, A)
        in_maps.append(m)
    return in_maps


def kernel(small, middle, large, pre_scale):
    global LAST_EXEC_NS, LAST_RESULTS
    small = np.asarray(small, dtype=np.float32)
    middle = np.asarray(middle, dtype=np.float32)
    large = np.asarray(large, dtype=np.float32)
    in_maps = _make_in_maps(small, middle, large, pre_scale)
    nc = _get_program()
    res = run_bass_kernel_spmd(nc, in_maps, list(range(N_CORES)))
    LAST_EXEC_NS = res.exec_time_ns
    LAST_RESULTS = res
    chunks = []
    for nm, S, rows, dect, P, K, T, A in LEVELS:
        cells = rows * S * 3
        for c in range(N_CORES):
            o = np.asarray(res.results[c][f"o_{nm}"])[:cells]
            chunks.append(o.astype(np.float32))
    return np.concatenate(chunks, axis=0)


# revision 11
# speedup vs baseline: 1.9051x; 1.0280x over previous
"""YOLO-head decode (nms_detection) Bass kernel for 8 trn2 NeuronCores.

Reference computation per pyramid level p [S, S, 3, 85]:
  conf = p[...,0]
  x = (sigmoid(p[...,1]) + i) / S        (i = index along FIRST spatial axis)
  y = (sigmoid(p[...,2]) + j) / S
  w = exp(p[...,3]) * anchor_w           (anchor = pre_scale[dect]/416)
  h = exp(p[...,4]) * anchor_h
  lix = argmax(p[...,5:85])  (first-max tie-break)
  row = [x,y,w,h,lix,conf] * (conf > 0.5)
Output = concat over levels of rows, [681408, 6].

Sharding: each level split along its leading spatial axis into 8 row-shards
(104->13, 208->26, 416->52 rows per core). Decode is elementwise per cell, so
cores are fully independent; host concatenates the per-core outputs.

Layout: cells on partitions, K cells per partition per tile, padded per level
so all 128 partitions are used (small 1x128x32, middle 2x128x64, large
13x128x39; pad cells are zeros -> conf 0 -> masked, host drops them). Grid
x/y offsets and anchors are per-(tile, partition, k) constants computed on
host into small per-level tables (no per-cell aux tensor DMA).

argmax via value/index packing so one max-reduce yields both:
  ACT:  t   = Identity(v * 2^14 + 1.5*2^23)      -> fp32 magic-round, t = M + r
  DVE:  key = (t - M) + ((79-c)/128 - 0.3125)    -> r + (m-40)/128, m = 79-c
  DVE:  kmax = reduce_max(key)  (single InstTensorReduce over the 80 classes)
  ACT:  t2  = Identity(kmax + M)                 -> M + r*
  DVE:  e   = (t2 - M) - kmax = -(m*-40)/128 ;  lix = 128*e + 39 = c*
All steps are exact in fp32 for |v| < 8 (keys are integers r plus exact
multiples of 1/128, |r| <= 2^17), so the only deviation from jnp.argmax is
the 2^-14 quantization of logits (ties broken toward the first index, same
as argmax; measured rel-err contribution ~3.7e-3, well under the 2e-2 gate).

sigmoid is computed as 1/(1+exp(-x)) (ACT Exp with scale=-1 + Identity(+1),
DVE reciprocal_approx_fast) because no ACT table set holds both sigmoid and
exp -- a native Sigmoid would force two 1.3us table reloads per tile.

Outputs are written bf16 (halves output DMA; x/y/w/h/conf tolerate the 2^-9
rounding, lix <= 79 is exact in bf16) and converted to fp32 on host.
"""

import os
import sys
from contextlib import ExitStack

import numpy as np

for _p in ("/root/.axon_site/_ro/trn_rl_repo", "/opt/trn_rl_repo"):
    if os.path.isdir(_p) and _p not in sys.path:
        sys.path.append(_p)

import concourse.bacc as bacc
import concourse.bass as bass
import concourse.tile as tile
import concourse.mybir as mybir
from concourse.bass_utils import run_bass_kernel_spmd

F32 = mybir.dt.float32
BF16 = mybir.dt.bfloat16
Alu = mybir.AluOpType
Act = mybir.ActivationFunctionType
AxX = mybir.AxisListType.X

N_CORES = 8
MAGIC = 12582912.0  # 1.5 * 2^23: float + MAGIC - MAGIC == round-to-nearest-int
QS = 16384.0  # 2^14 logit quantization scale

# (name, S, rows_per_core, dect_size, P, K, T, A)
#   P*K*T >= rows*S*3 (pad), A = number of distinct anchor k-patterns over t
LEVELS = [
    ("small", 104, 13, 3, 128, 32, 1, 1),
    ("middle", 208, 26, 4, 128, 64, 2, 2),
    ("large", 416, 52, 5, 128, 39, 13, 1),
]

LAST_EXEC_NS = None
LAST_RESULTS = None

_prog_cache = {}


def _build_program():
    nc = bacc.Bacc(trn_type="TRN2")
    xins, csts, outs = {}, {}, {}
    for nm, S, rows, dect, P, K, T, A in LEVELS:
        Ncap = P * K * T
        W = (T + A) * 2 * K
        xins[nm] = nc.dram_tensor(f"x_{nm}", [Ncap, 85], F32, kind="ExternalInput")
        csts[nm] = nc.dram_tensor(f"c_{nm}", [128, W], F32, kind="ExternalInput")
        outs[nm] = nc.dram_tensor(f"o_{nm}", [Ncap, 6], BF16, kind="ExternalOutput")
    # cols 0:80 = (79-c)/128 - 0.3125 ; col 80 = MAGIC (activation bias AP)
    iot = nc.dram_tensor("iota", [128, 81], F32, kind="ExternalInput")

    with tile.TileContext(nc) as tc, ExitStack() as ctx:
        const = ctx.enter_context(tc.tile_pool(name="const", bufs=1))
        pin_p = ctx.enter_context(tc.tile_pool(name="pin", bufs=4))
        qt_p = ctx.enter_context(tc.tile_pool(name="qtp", bufs=2))
        km_p = ctx.enter_context(tc.tile_pool(name="kmp", bufs=2))
        sml_p = ctx.enter_context(tc.tile_pool(name="sml", bufs=3))
        out_p = ctx.enter_context(tc.tile_pool(name="outp", bufs=3))

        it = const.tile([128, 81], F32)
        nc.sync.dma_start(it[:], iot[:])
        ctiles = {}
        for nm, S, rows, dect, P, K, T, A in LEVELS:
            ct = const.tile([128, (T + A) * 2 * K], F32)
            nc.sync.dma_start(ct[:], csts[nm][:])
            ctiles[nm] = ct

        magic_ap = it[0:128, 80:81]
        for nm, S, rows, dect, P, K, T, A in LEVELS:
            inv = float(np.float32(1.0 / S))
            xv = xins[nm][:].rearrange("(t p k) c -> t p (k c)", p=P, k=K)
            ov = outs[nm][:].rearrange("(t p k) c -> t p (k c)", p=P, k=K)
            ct = ctiles[nm]
            iota_b = (
                it[0:P, 0:80]
                .rearrange("p (o c) -> p o c", o=1)
                .broadcast_to([P, K, 80])
            )
            for t in range(T):
                gxy_v = ct[0:P, t * 2 * K : (t + 1) * 2 * K].rearrange(
                    "p (k c) -> p k c", c=2
                )
                ao = (T + (t % A)) * 2 * K
                awh_v = ct[0:P, ao : ao + 2 * K].rearrange("p (k c) -> p k c", c=2)

                pin = pin_p.tile([P, K * 85], F32, tag="pin")
                nc.sync.dma_start(pin[:], xv[t])
                pv = pin[:].rearrange("p (k c) -> p k c", c=85)

                ot = out_p.tile([P, K * 6], BF16, tag="out")
                ovv = ot[:].rearrange("p (k c) -> p k c", c=6)

                # ACT: magic-round first (it gates DVE's big ops), then
                # exp(-txy) for sigmoid, exp(twh), conf copy
                qt = qt_p.tile([P, K * 80], F32, tag="qt")
                qv = qt[:].rearrange("p (k c) -> p k c", c=80)
                nc.scalar.activation(
                    qv, pv[:, :, 5:85], Act.Identity, bias=magic_ap, scale=QS
                )
                es = sml_p.tile([P, K * 2], F32, tag="es")
                esv = es[:].rearrange("p (k c) -> p k c", c=2)
                nc.scalar.activation(esv, pv[:, :, 1:3], Act.Exp, scale=-1.0)
                sp = sml_p.tile([P, K * 2], F32, tag="sp")
                spv = sp[:].rearrange("p (k c) -> p k c", c=2)
                nc.scalar.activation(spv, esv, Act.Identity, bias=1.0)
                ext = sml_p.tile([P, K * 2], F32, tag="exp")
                exv = ext[:].rearrange("p (k c) -> p k c", c=2)
                nc.scalar.activation(exv, pv[:, :, 3:5], Act.Exp)
                nc.scalar.copy(ovv[:, :, 5:6], pv[:, :, 0:1])

                # DVE: key = (t - M) + iota'   (in place over qt)
                nc.vector.scalar_tensor_tensor(qv, qv, -MAGIC, iota_b, Alu.add, Alu.add)

                # DVE: packed max over the 80 classes in one reduce
                km = km_p.tile([P, K], F32, tag="km")
                nc.vector.tensor_reduce(km[:], qv, axis=AxX, op=Alu.max)

                # DVE: t2 = M + r* (fp32 write rounds); e = (t2 - M) - kmax;
                # lix = 128*e + 39 (exact int)
                t2 = sml_p.tile([P, K], F32, tag="t2")
                nc.vector.tensor_scalar(t2[:], km[:], 1.0, MAGIC, Alu.mult, Alu.add)
                e = sml_p.tile([P, K], F32, tag="e")
                nc.vector.scalar_tensor_tensor(
                    e[:], t2[:], -MAGIC, km[:], Alu.add, Alu.subtract
                )
                ev = e[:].rearrange("p (k c) -> p k c", c=1)
                nc.vector.tensor_scalar(
                    ovv[:, :, 4:5], ev, 128.0, 39.0, Alu.mult, Alu.add
                )

                # DVE: sigmoid = 1/(1+exp(-x)); (x,y) = sig*inv + (gx,gy);
                # (w,h) = exp * (aw,ah); mask = conf > 0.5; zero failing rows
                sg = sml_p.tile([P, K * 2], F32, tag="sg")
                nc.vector.reciprocal_approx_fast(sg[:], sp[:])
                sgv = sg[:].rearrange("p (k c) -> p k c", c=2)
                nc.vector.scalar_tensor_tensor(
                    ovv[:, :, 0:2], sgv, inv, gxy_v, Alu.mult, Alu.add
                )
                nc.vector.scalar_tensor_tensor(
                    ovv[:, :, 2:4], exv, 1.0, awh_v, Alu.mult, Alu.mult
                )
                mk = sml_p.tile([P, K], BF16, tag="mk")
                nc.vector.tensor_scalar(mk[:], pv[:, :, 0], 0.5, None, Alu.is_gt)
                mk_b = mk[:].rearrange("p (k o) -> p k o", o=1).broadcast_to([P, K, 6])
                nc.vector.scalar_tensor_tensor(ovv, ovv, 1.0, mk_b, Alu.mult, Alu.mult)

                nc.sync.dma_start(ov[t], ot[:])
    nc.compile()
    return nc


def _get_program():
    if "nc" not in _prog_cache:
        _prog_cache["nc"] = _build_program()
    return _prog_cache["nc"]


def _make_cst(core, ps, S, rows, dect, P, K, T, A):
    """Per-level const table [128, (T+A)*2K]: per-tile (gx,gy) | awh variants."""
    inv = np.float32(1.0 / S)
    anc = (ps[dect] / np.float32(416.0)).astype(np.float32)  # [3, 2]
    cells = rows * S * 3
    Ncap = P * K * T
    base = core * cells
    g = base + np.minimum(np.arange(Ncap), cells - 1)
    arr = g.reshape(T, P, K)
    i = arr // (S * 3)
    j = (arr % (S * 3)) // 3
    aa = arr % 3
    cst = np.zeros((128, (T + A) * 2 * K), np.float32)
    blk = np.empty((P, K, 2), np.float32)
    for t in range(T):
        blk[..., 0] = i[t].astype(np.float32) * inv
        blk[..., 1] = j[t].astype(np.float32) * inv
        cst[:P, t * 2 * K : (t + 1) * 2 * K] = blk.reshape(P, 2 * K)
    for va in range(A):
        blk[..., 0] = anc[aa[va], 0]
        blk[..., 1] = anc[aa[va], 1]
        cst[:P, (T + va) * 2 * K : (T + va + 1) * 2 * K] = blk.reshape(P, 2 * K)
    return cst


def _make_in_maps(small, middle, large, pre_scale):
    arrs = {"small": small, "middle": middle, "large": large}
    ps = np.asarray(pre_scale, dtype=np.float32)
    iota = np.empty((128, 81), np.float32)
    iota[:, 0:80] = (79.0 - np.arange(80)).astype(np.float32) / np.float32(
        128.0
    ) - np.float32(0.3125)
    iota[:, 80] = MAGIC
    in_maps = []
    for c in range(N_CORES):
        m = {"iota": iota}
        for nm, S, rows, dect, P, K, T, A in LEVELS:
            cells = rows * S * 3
            Ncap = P * K * T
            x = np.asarray(arrs[nm][c * rows : (c + 1) * rows], dtype=np.float32)
            xp = np.zeros((Ncap, 85), np.float32)
            xp[:cells] = x.reshape(cells, 85)
            m[f"x_{nm}"] = xp
            m[f"c_{nm}"] = _make_cst(c, ps, S, rows, dect, P, K, T, A)
        in_maps.append(m)
    return in_maps


def kernel(small, middle, large, pre_scale):
    global LAST_EXEC_NS, LAST_RESULTS
    small = np.asarray(small, dtype=np.float32)
    middle = np.asarray(middle, dtype=np.float32)
    large = np.asarray(large, dtype=np.float32)
    in_maps = _make_in_maps(small, middle, large, pre_scale)
    nc = _get_program()
    res = run_bass_kernel_spmd(nc, in_maps, list(range(N_CORES)))
    LAST_EXEC_NS = res.exec_time_ns
    LAST_RESULTS = res
    chunks = []
    for nm, S, rows, dect, P, K, T, A in LEVELS:
        cells = rows * S * 3
        for c in range(N_CORES):
            o = np.asarray(res.results[c][f"o_{nm}"])[:cells]
            chunks.append(o.astype(np.float32))
    return np.concatenate(chunks, axis=0)
